# revision 76
# baseline (speedup 1.0000x reference)
"""Trainium2 Bass kernel for nn_DPConv_16011638080100.

8 NeuronCores, data-parallel over batch N=16 -> 2 samples/core.

Per core (vs the bf16 baseline):
  - conv1a folded into the 9 pos/conv2 taps host-side (tap' = 0.25*tap@W1a),
    so pooling writes pooled-x window tiles directly and conv1/psA vanish.
  - pooled-x stored as fp8 hi + fp8 residual lo (hi: Act convert, lo: gpsimd
    subtract); the 3x3+1x1 conv runs as fp8 DoubleRow matmuls: each tap is
    one DR matmul whose two k-tiles multiply the same fp8 weights against
    the hi and lo data (full data compensation), plus one extra DR matmul
    compensating the two largest weight-quantization residuals.
    DR runs at 0.5 cycles/row -> ~4x fewer PE rows than bf16 taps.
  - taps use a flat-conv formulation: the padded 34x34 window rows are read
    as flat runs (12/12/8 row blocks, <=512 psum cols incl. discarded
    pad-straddle outputs) so the DR rhs AP is [64p, 2, N] as the ISA needs.
  - SE lagged one round behind taps so PE never stalls on the Act eviction.
  - fold via gd DRAM round trip + 3-way p-split bulk G reload (per-round
    narrow reloads lose: DMA cost scales with bytes PER PARTITION).
  - conv3: bf16 G path + bf16 X path, X matmuls prefilled into psum during
    the G reload; output written bf16 with contiguous 8KB DMA runs.
"""
import sys

sys.path.insert(0, "/opt/trn_rl_repo")

import numpy as np

N_CORES = 8
C = 64
H = W = 128
STAP = 64.0
S3 = 64.0

# pooling segments (out_start, count, src_start); same table for H and W
PAIR_SEGS = [(4, 12, 0), (16, 16, 13), (32, 16, 30), (48, 16, 47),
             (64, 16, 64), (80, 16, 81), (96, 16, 98), (112, 12, 115)]
COPY_SEGS = [(0, 4, 0), (124, 4, 127)]  # emitted as self-adds

# flat-conv row blocks: (r0, nrows, flat_out_len, psum_col_off)
# row stride 33: 1 shared pad col between rows (left pad of row r+1 is the
# right pad of row r); 1 top + 1 bottom pad row; region padded to 1124 so
# f32 bitcasts stay aligned.
RSTR = 33
REG = 1124
BLOCKS = [(0, 12, 11 * RSTR + 32, 0), (12, 12, 11 * RSTR + 32, 512),
          (24, 8, 7 * RSTR + 32, 1024)]

TAPS = [(kh, kw) for kh in range(3) for kw in range(3)]

# fp8 weight pack columns: 10 "taps" x 2 window-positions x 2 k-tiles x 128
# ([W|0] for window 0, [0|W] for window 1: DoubleRow outputs must sit at
# psum partition base 0, so each DR writes all 128 partitions and the
# zero half accumulates nothing); tap 9 = center weight residual
WT_O = 0
NW8 = 10 * 512
# bf16 pack: W3a*S3, W3bb (x2 path)
W3A_O = 0
W3BB_O = 64
NWB = 128
# f32 pack (SE weights + biases)
SE1_O = 0
SE2_O = 8
BPR_O = SE2_O + 64
B3_O = BPR_O + 1
SVC_O = B3_O + 1
NWF = SVC_O + 1

_cache = {}


def _segs(lo, hi):
    out = []
    for (o, c, s) in PAIR_SEGS:
        if lo <= o and o + c <= hi:
            out.append((o, c, s, False))
    for (o, c, s) in COPY_SEGS:
        if lo <= o and o + c <= hi:
            out.append((o, c, s, True))
    return out


def build_program(has_corr):
    """has_corr: whether the conv1-bias border correction path is built."""
    import concourse.bass as bass
    import concourse.tile as tile
    import concourse.mybir as mybir
    from concourse.ap import AP
    from concourse import bacc
    from contextlib import ExitStack

    f32 = mybir.dt.float32
    bf16 = mybir.dt.bfloat16
    fp8 = mybir.dt.float8e4
    Alu = mybir.AluOpType
    Act = mybir.ActivationFunctionType
    DR = mybir.MatmulPerfMode.DoubleRow

    nc = bacc.Bacc("TRN2", target_bir_lowering=False, debug=False)
    xs_d = nc.dram_tensor("xs", [2, C, H, W], bf16, kind="ExternalInput").ap()
    wt8_d = nc.dram_tensor("wt8", [128, NW8], fp8, kind="ExternalInput").ap()
    wsb_d = nc.dram_tensor("wsb16", [128, NWB], bf16, kind="ExternalInput").ap()
    wsf_d = nc.dram_tensor("wsf", [128, NWF], f32, kind="ExternalInput").ap()
    corr_d = None
    if has_corr:
        corr_d = nc.dram_tensor("corr", [128, 1024], f32,
                                kind="ExternalInput").ap()
    out_d = nc.dram_tensor("out", [2, C, H, W], bf16, kind="ExternalOutput").ap()

    with tile.TileContext(nc) as tc, ExitStack() as ctx:
        persist = ctx.enter_context(tc.tile_pool(name="persist", bufs=1))
        xh_p = ctx.enter_context(tc.tile_pool(name="xh", bufs=2))
        pxb_p = ctx.enter_context(tc.tile_pool(name="pxb", bufs=3))
        prs_p = ctx.enter_context(tc.tile_pool(name="prs", bufs=3))
        att_p = ctx.enter_context(tc.tile_pool(name="att", bufs=2))
        sml_p = ctx.enter_context(tc.tile_pool(name="sml", bufs=4))

        gd_p = ctx.enter_context(tc.tile_pool(name="gdp", bufs=1, space="DRAM"))
        gd = gd_p.tile([16 * 128, 1024], bf16)

        wsf = persist.tile([128, NWF], f32)
        wsb16 = persist.tile([128, NWB], bf16)
        wt8 = persist.tile([128, NW8], fp8)
        X = persist.tile([128, H * W], bf16)
        G = persist.tile([128, H * W], bf16)
        warm = persist.tile([128, 1], f32)
        corr = persist.tile([128, 1024], f32) if has_corr else None
        p8_tiles = [persist.tile([128, 2 * 2 * REG], fp8, tag=f"p8{k}",
                                 name=f"p8{k}") for k in range(3)]
        X3 = X.rearrange("z (h w) -> z h w", h=H)

        nc.scalar.dma_start(out=wt8[:, 0:1536], in_=wt8_d[:, 0:1536])
        nc.scalar.dma_start(out=wt8[:, 1536:NW8], in_=wt8_d[:, 1536:NW8])
        nc.scalar.dma_start(out=wsf, in_=wsf_d)
        nc.scalar.dma_start(out=wsb16, in_=wsb_d)
        if has_corr:
            nc.scalar.dma_start(out=corr, in_=corr_d)
        xs_f = xs_d.rearrange("s c h w -> (s c) (h w)")
        x_blocks = [(0, 16, nc.sync), (16, 30, nc.gpsimd),
                    (30, 64, nc.sync), (64, 98, nc.scalar),
                    (98, 128, nc.sync)]
        for r0, r1, q in x_blocks:
            q.dma_start(out=X[:, r0 * W:r1 * W], in_=xs_f[:, r0 * W:r1 * W])

        with tc.tile_pool(name="psB", bufs=2, space="PSUM") as psB, \
             tc.tile_pool(name="psE", bufs=1, space="PSUM") as psE:

            def tiny_mm(one, dep_ap):
                """single-wait absorber: PE observes dep_ap's producer(s)."""
                scr = psE.tile([128, 1024], f32, tag="e")
                if one.dtype != f32:
                    one = one.bitcast(f32)
                if dep_ap.dtype != f32:
                    dep_ap = dep_ap.bitcast(f32)
                n = dep_ap.free_size()
                nc.tensor.matmul(scr[0:1, 0:n], one, dep_ap,
                                 start=True, stop=True)

            nc.scalar.activation(out=warm[0:1, 0:1], in_=wsf[0:1, 0:1],
                                 func=Act.Sigmoid)
            nc.scalar.activation(out=warm[0:1, 0:1], in_=warm[0:1, 0:1],
                                 func=Act.Relu)
            tiny_mm(wsf[0:1, 0:1], wsf[0:1, 0:1])
            tiny_mm(wsf[0:1, 0:1], wsb16[0:1, 0:2].bitcast(f32))
            tiny_mm(wsf[0:1, 0:1], wt8[0:1, 0:4].bitcast(f32))

            units = {}
            n_pool = [0]

            def do_pool(i, hb):
                """pool unit (i, hb): 32 pooled H rows of block i, W half hb
                -> padded fp8 hi/lo tile [128, 2*2*1156]."""
                xh = xh_p.tile([128, 32 * 64], bf16, tag="xh")
                xh3 = xh.rearrange("z (h w) -> z h w", h=32)
                for (o, cnt, s, cp) in _segs(32 * i, 32 * i + 32):
                    ol = o - 32 * i
                    if cp:
                        src = X3[:, s:s + 1, hb * 64:hb * 64 + 64]
                        src = src.broadcast_to((128, cnt, 64))
                        in0 = in1 = src
                    else:
                        in0 = X3[:, s:s + cnt, hb * 64:hb * 64 + 64]
                        in1 = X3[:, s + 1:s + 1 + cnt, hb * 64:hb * 64 + 64]
                    nc.gpsimd.tensor_tensor(
                        out=xh3[:, ol:ol + cnt, :], in0=in0, in1=in1,
                        op=Alu.add)
                pxb = pxb_p.tile([128, 2048], bf16, tag="pxb")
                px4 = pxb.rearrange("z (l h w) -> z l h w", l=2, h=32)
                for (o, cnt, s, cp) in _segs(64 * hb, 64 * hb + 64):
                    jloc = (o - 64 * hb) // 32
                    w0 = (o - 64 * hb) % 32
                    sl = s - 64 * hb
                    if cp:
                        src = xh3[:, :, sl:sl + 1].broadcast_to((128, 32, cnt))
                        in0 = in1 = src
                    else:
                        in0 = xh3[:, :, sl:sl + cnt]
                        in1 = xh3[:, :, sl + 1:sl + 1 + cnt]
                    nc.vector.tensor_tensor(
                        out=px4[:, jloc, :, w0:w0 + cnt], in0=in0, in1=in1,
                        op=Alu.add)
                p8 = p8_tiles[n_pool[0] % 3]
                if n_pool[0] < 3:
                    nc.gpsimd.memset(p8.bitcast(f32), 0.0)
                n_pool[0] += 1
                pap = list(p8.ap[0])
                hi_int = AP(p8.tensor, p8.offset + RSTR + 1,
                            [pap, [REG, 2], [RSTR, 32], [1, 32]])
                lo_int = AP(p8.tensor, p8.offset + 2 * REG + RSTR + 1,
                            [pap, [REG, 2], [RSTR, 32], [1, 32]])
                # hi = fp8(pxb)
                nc.scalar.activation(out=hi_int, in_=px4, func=Act.Identity)
                # lo = fp8(pxb - hi)
                nc.gpsimd.tensor_tensor(out=lo_int, in0=px4, in1=hi_int,
                                        op=Alu.subtract)
                return p8

            # 10 taps: the 9 conv taps + the center weight-residual "tap"
            TAPS10 = TAPS + [(1, 1)]

            def do_taps(s, t):
                """10 DR matmuls per (window, block): each computes
                W_k @ (hi + lo) at the tap's shift."""
                p8 = units[(t // 2, t % 2)]
                sb = s * 64
                p8s = p8[sb:sb + 64, :]
                pap = list(p8s.ap[0])
                prp = psB.tile([128, 1536], f32, tag="b")
                for (r0, nr, olen, po) in BLOCKS:
                    o = prp[:, po:po + olen]
                    for l in range(2):
                        for idx, (kh, kw) in enumerate(TAPS10):
                            c0 = idx * 512 + l * 256
                            lhsT = wt8[sb:sb + 64, c0:c0 + 256]
                            lhsT = lhsT.rearrange("z (two f) -> z two f",
                                                  two=2)
                            off = l * REG + (r0 + kh) * RSTR + kw
                            rhs = AP(p8s.tensor, p8s.offset + off,
                                     [pap, [2 * REG, 2], [1, olen]])
                            nc.tensor.matmul(o, lhsT, rhs,
                                             start=(l == 0 and idx == 0),
                                             stop=(l == 1 and idx == 9),
                                             perf_mode=DR,
                                             skip_group_check=True)
                return prp

            def do_evict(s, t, prp):
                """psum -> prs (f32) with bias+1/STAP, SE sum via accum."""
                prs = prs_p.tile([128, 1024], f32, tag="prs")
                sv2 = sml_p.tile([128, 2], f32, tag="sv2")
                prr = prs.rearrange("z (r w) -> z r w", r=32)
                src01 = AP(prp.tensor, prp.offset,
                           [list(prp.ap[0]), [512, 2], [RSTR, 12], [1, 32]])
                nc.scalar.activation(out=prr[:, 0:24, :], in_=src01,
                                     func=Act.Identity,
                                     bias=wsf[:, BPR_O:BPR_O + 1],
                                     scale=1.0 / STAP,
                                     accum_out=sv2[:, 0:1])
                src2 = AP(prp.tensor, prp.offset + 1024,
                          [list(prp.ap[0]), [RSTR, 8], [1, 32]])
                nc.scalar.activation(out=prr[:, 24:32, :], in_=src2,
                                     func=Act.Identity,
                                     bias=wsf[:, BPR_O:BPR_O + 1],
                                     scale=1.0 / STAP,
                                     accum_out=sv2[:, 1:2])
                svec = sml_p.tile([128, 1], f32, tag="sv")
                if has_corr:
                    prs2 = prs_p.tile([128, 1024], f32, tag="prs2")
                    nc.vector.tensor_tensor(out=prs2, in0=prs, in1=corr,
                                            op=Alu.add)
                    prs = prs2
                    nc.vector.tensor_scalar(
                        out=svec[:, 0:1], in0=sv2[:, 0:1],
                        scalar1=sv2[:, 1:2], scalar2=wsf[:, SVC_O:SVC_O + 1],
                        op0=Alu.add, op1=Alu.add)
                else:
                    nc.vector.tensor_tensor(out=svec[:, 0:1], in0=sv2[:, 0:1],
                                            in1=sv2[:, 1:2], op=Alu.add)
                return prs, svec

            def do_tail(s, t, prs, svec):
                """SE mlp + att + gd dump + per-round G reload (lagged)."""
                se1 = psE.tile([128, 1024], f32, tag="e")
                for par in range(2):
                    pb = par * 64
                    nc.tensor.matmul(se1[0:8, par * 512:par * 512 + 1],
                                     wsf[pb:pb + 64, SE1_O:SE1_O + 8],
                                     svec[pb:pb + 64, 0:1],
                                     start=True, stop=True)
                s1sb = sml_p.tile([128, 1], f32, tag="s1")
                for par in range(2):
                    pb = par * 64
                    nc.scalar.activation(out=s1sb[pb:pb + 8, 0:1],
                                         in_=se1[0:8, par * 512:par * 512 + 1],
                                         func=Act.Relu)
                se2 = psE.tile([128, 1024], f32, tag="e")
                for par in range(2):
                    pb = par * 64
                    nc.tensor.matmul(se2[0:64, par * 512:par * 512 + 1],
                                     wsf[pb:pb + 8, SE2_O:SE2_O + 64],
                                     s1sb[pb:pb + 8, 0:1],
                                     start=True, stop=True)
                s2sb = sml_p.tile([128, 1], f32, tag="s2")
                for par in range(2):
                    pb = par * 64
                    nc.scalar.activation(out=s2sb[pb:pb + 64, 0:1],
                                         in_=se2[0:64, par * 512:par * 512 + 1],
                                         func=Act.Sigmoid)
                sp = sml_p.tile([128, 1], f32, tag="sp")
                nc.vector.tensor_scalar_add(sp[:, 0:1], s2sb[:, 0:1], 1.0)
                att = att_p.tile([128, 1024], bf16, tag="att")
                nc.vector.tensor_scalar(out=att, in0=prs, scalar1=sp[:, 0:1],
                                        scalar2=None, op0=Alu.mult)
                k = s * 8 + t
                nc.sync.dma_start(out=gd[k * 128:k * 128 + 128, :], in_=att)

            seq = [(s, t) for t in range(8) for s in (0, 1)]
            # keep PE at full p-state through the X load + first pool-chain
            # latency: dummy DR matmuls gated only on the wt8 weight load
            for _ in range(72):
                scr = psE.tile([128, 1024], f32, tag="e")
                lhsT = wt8[0:64, 0:128].rearrange("z (two f) -> z two f",
                                                  two=2)
                rhs = AP(wt8.tensor, wt8.offset,
                         [[list(wt8.ap[0])[0], 64], [448, 2], [1, 448]])
                nc.tensor.matmul(scr[0:64, 0:448], lhsT, rhs,
                                 start=True, stop=True, perf_mode=DR)
            units[(0, 0)] = do_pool(0, 0)
            units[(0, 1)] = do_pool(0, 1)
            units[(1, 0)] = do_pool(1, 0)
            pending = None
            for kk in range(17):
                if kk < 16:
                    s, t = seq[kk]
                    if s == 0:
                        nxt = (t + 2) // 2, (t + 2) % 2
                        if t + 2 <= 7 and nxt not in units:
                            units[nxt] = do_pool(*nxt)
                    u = (t // 2, t % 2)
                    # absorb pool-chain sems into PE stream
                    pa = list(units[u].ap[0])
                    dep = AP(units[u].tensor, units[u].offset + RSTR + 3,
                             [[pa[0], 1], [2 * REG, 2], [1, 4]])
                    tiny_mm(wsf[0:1, 0:1], dep)
                    prp = do_taps(s, t)
                    prs, svec = do_evict(s, t, prp)
                if pending is not None:
                    do_tail(*pending)
                if kk < 16:
                    pending = (s, t, prs, svec)
            # absorb the X-chunk DMA sems into the PE stream (conv3's X
            # matmuls read X; the chunks landed long ago so these are free)
            for r0, _, _q in x_blocks:
                tiny_mm(X[0:1, r0 * W:r0 * W + 2], X[0:1, r0 * W:r0 * W + 2])

        # reload G from DRAM: G[z=k*8+pb, p*1024+w] = gd[(k*128+pb*16+p), w];
        # split along p across the three DMA-capable queues.
        gd3 = gd.rearrange("(k pb p) w -> (k pb) p w", pb=8, p=16)
        for (p0, p1), eng in (((0, 5), nc.sync), ((5, 10), nc.scalar),
                              ((10, 16), nc.gpsimd)):
            dst = G[:, p0 * 1024:p1 * 1024].rearrange(
                "z (p w) -> z p w", p=p1 - p0)
            eng.dma_start(out=dst, in_=gd3[:, p0:p1, :])

        # ---- conv3 ----
        Xr = X.rearrange("z (i p q w) -> z i p q w", i=4, p=16, q=2)
        Gr = G.rearrange("z (p q r sl ij) -> z p q r sl ij",
                         p=16, q=2, r=16, sl=2)
        od = out_d.rearrange("s c (i pq) w -> (s c) i (pq w)", i=4)
        with tc.tile_pool(name="psC", bufs=4, space="PSUM") as psC, \
             tc.tile_pool(name="outp", bufs=2) as out_p:
            # groups: (i, q, jh); each [128, 1024] psum (2 banks, 4 live)
            groups = [(i, q, jh) for i in range(4) for q in range(2)
                      for jh in range(2)]
            pcs = {}
            ots = {}

            def xdr(g):
                i, q, jh = g
                pc = psC.tile([128, 1024], f32, tag="c",
                              name=f"pc{i}{q}{jh}")
                pcs[g] = pc
                for s in range(2):
                    sb = s * 64
                    for jj in range(2):
                        j = 2 * jh + jj
                        o = pc[sb:sb + 64, jj * 512:jj * 512 + 512]
                        rhsx = Xr[sb:sb + 64, i, :, q, 32 * j:32 * j + 32]
                        nc.tensor.matmul(
                            o, wsb16[sb:sb + 64, W3BB_O:W3BB_O + 64],
                            rhsx, start=True, stop=False,
                            skip_group_check=True)

            def gmm_evict(g, n):
                i, q, jh = g
                pc = pcs[g]
                for s in range(2):
                    sb = s * 64
                    for jj in range(2):
                        j = 2 * jh + jj
                        o = pc[sb:sb + 64, jj * 512:jj * 512 + 512]
                        rhsg = Gr[sb:sb + 64, :, q, :, :, 4 * i + j]
                        nc.tensor.matmul(
                            o, wsb16[sb:sb + 64, W3A_O:W3A_O + 64],
                            rhsg, start=False, stop=True,
                            skip_group_check=True)
                if i not in ots:
                    ots[i] = out_p.tile([128, 4096], bf16, tag="o",
                                        name=f"ot{i}")
                ot4 = ots[i].rearrange("z (p q w) -> z p q w", p=16, q=2)
                dst = ot4[:, :, q, :].rearrange(
                    "z p (j r sl) -> z j p r sl", j=4, r=16)[:, 2 * jh:2 * jh + 2]
                srcf = pc.rearrange(
                    "z (j p r sl) -> z j p r sl", j=2, p=16, r=16)
                if n % 2 == 0:
                    nc.scalar.activation(
                        out=dst, in_=srcf, func=Act.Identity,
                        bias=wsf[:, B3_O:B3_O + 1], scale=1.0 / S3)
                else:
                    nc.vector.tensor_scalar(
                        out=dst, in0=srcf, scalar1=1.0 / S3,
                        scalar2=wsf[:, B3_O:B3_O + 1],
                        op0=Alu.mult, op1=Alu.add)

            # prefill 4 groups' X-DRs (independent of the G reload), then per
            # group: G matmuls + evict, issuing the next group's X-DRs as
            # psum buffers free
            for g in groups[:4]:
                xdr(g)
            for n, g in enumerate(groups):
                gmm_evict(g, n)
                if n + 4 < len(groups):
                    xdr(groups[n + 4])
                i, q, jh = g
                if q == 1 and jh == 1:
                    nc.sync.dma_start(out=od[:, i, 0:2048],
                                      in_=ots[i][:, 0:2048])
                    nc.gpsimd.dma_start(out=od[:, i, 2048:4096],
                                        in_=ots[i][:, 2048:4096])

    nc.compile()
    return nc


def _prep_inputs(inputs):
    import ml_dtypes as mld

    FP8 = mld.float8_e4m3
    BF16 = mld.bfloat16
    q8 = lambda a: np.asarray(a, np.float32).astype(FP8)

    x = np.asarray(inputs["x"], np.float32)
    w1 = np.asarray(inputs["conv1_w"], np.float32)[:, :, 0, 0]
    b1 = np.asarray(inputs["conv1_b"], np.float32)
    w2 = np.asarray(inputs["conv2_w"], np.float32)[:, :, 0, 0]
    b2 = np.asarray(inputs["conv2_b"], np.float32)
    w3 = np.asarray(inputs["conv3_w"], np.float32)[:, :, 0, 0]
    b3 = np.asarray(inputs["conv3_b"], np.float32)
    pw = np.asarray(inputs["pos_w"], np.float32)
    pb = np.asarray(inputs["pos_b"], np.float32)
    s1w = np.asarray(inputs["se_w1"], np.float32)[:, :, 0, 0]
    s2w = np.asarray(inputs["se_w2"], np.float32)[:, :, 0, 0]

    xs = np.ascontiguousarray(x.astype(BF16))

    W1a, W1b = w1[:64], w1[64:]
    W3a, W3b = w3[:, :64], w3[:, 64:]
    W3bb = W3b @ W1b
    b1a = b1[:64]

    def dup(dst, col, mat):
        dst[0:mat.shape[0], col:col + mat.shape[1]] = mat
        dst[64:64 + mat.shape[0], col:col + mat.shape[1]] = mat

    # tap weights: hi + residuals
    whis, wres = {}, {}
    for (kh, kw) in TAPS:
        t = pw[:, :, kh, kw]
        if kh == 1 and kw == 1:
            t = t + w2
        ws = 0.25 * (t @ W1a) * STAP
        hi = q8(ws)
        whis[(kh, kw)] = hi
        wres[(kh, kw)] = ws - hi.astype(np.float32)

    wt8 = np.zeros((128, NW8), FP8)
    allw = [whis[k] for k in TAPS] + [q8(wres[(1, 1)])]
    for idx, wmat in enumerate(allw):
        for l in range(2):
            for kt in range(2):
                dup(wt8, idx * 512 + l * 256 + kt * 128 + l * 64, wmat.T)

    wsb = np.zeros((128, NWB), BF16)
    dup(wsb, W3A_O, (W3a * S3).T.astype(BF16))
    dup(wsb, W3BB_O, (W3bb * S3).T.astype(BF16))

    # per-tap bias contribution: the reference applies tap_ref @ (conv1 out),
    # whose +b1a term the folded taps drop for every interior tap position.
    tap_b1 = {}
    for (kh, kw) in TAPS:
        t = pw[:, :, kh, kw]
        if kh == 1 and kw == 1:
            t = t + w2
        tap_b1[(kh, kw)] = t @ b1a

    wsf = np.zeros((128, NWF), np.float32)
    dup(wsf, SE1_O, (s1w / 1024.0).T)
    dup(wsf, SE2_O, s2w.T)
    bpr = b2 + pb + sum(tap_b1.values())
    dup(wsf, BPR_O, bpr[:, None])
    dup(wsf, B3_O, (b3 + W3b @ b1[64:])[:, None])

    has_corr = bool(np.any(b1a != 0.0))
    corr = None
    if has_corr:
        # border defect: for output (r,c), taps whose padded source falls on
        # the ring contribute tap@b1a in the folded version but 0 in the
        # reference; subtract them (the all-interior sum is in bpr above).
        cimg = np.zeros((64, 32, 32), np.float32)
        for (kh, kw) in TAPS:
            v = tap_b1[(kh, kw)]
            rr = np.arange(32) + kh  # padded row idx = r+kh, ring if 0 or 33
            cc = np.arange(32) + kw
            ring = ((rr[:, None] == 0) | (rr[:, None] == 33)
                    | (cc[None, :] == 0) | (cc[None, :] == 33))
            cimg -= v[:, None, None] * ring[None, :, :]
        corr = np.zeros((128, 1024), np.float32)
        dup(corr, 0, cimg.reshape(64, 1024))
        svc = cimg.reshape(64, 1024).sum(axis=1)
        dup(wsf, SVC_O, svc[:, None])

    return xs, wt8, wsb, wsf, corr, has_corr


def kernel(**inputs):
    from concourse.bass_utils import run_bass_kernel_spmd

    xs, wt8, wsb, wsf, corr, has_corr = _prep_inputs(inputs)
    if _cache.get("key") != has_corr:
        _cache["nc"] = build_program(has_corr)
        _cache["key"] = has_corr
    nc = _cache["nc"]
    n = xs.shape[0]
    per = n // N_CORES
    in_maps = []
    for c in range(N_CORES):
        m = {"xs": xs[c * per:(c + 1) * per],
             "wt8": wt8, "wsb16": wsb, "wsf": wsf}
        if has_corr:
            m["corr"] = corr
        in_maps.append(m)
    res = run_bass_kernel_spmd(nc, in_maps, list(range(N_CORES)))
    _cache["last_res"] = res
    out = np.concatenate([res.results[c]["out"] for c in range(N_CORES)],
                         axis=0)
    return out.astype(np.float32)
